# revision 7
# baseline (speedup 1.0000x reference)
"""TRN2 Bass kernel for FFQLinear: y = x @ ((q - zp) * scale) + bias.

x: [2, 2048, 4096] f32, q: [4096, 4096] int32 (values 0..255),
scale/zero_point: [1] f32, bias: [4096] f32 -> y: [2, 2048, 4096] f32.

Strategy (8 NeuronCores, 2x4 grid):
  - M (=B*S=4096) split in 2 halves, N (=DOUT) split in 4 quarters.
  - Per core: x_shard [2048, 4096], q_shard [4096, 1024], out [2048, 1024].
  - Matmuls run in float32r (TF32-like, full PE rate, ~1e-4 precision).
    q values (integers 0..255) are exact under f32r truncation, so the
    zero-point is handled exactly via a row-sum correction:
      y = scale * (x @ q) - (scale*zp) * rowsum(x) + bias
  - x is transposed on-device through the PE (fp32 transpose via identity
    matmul), row-sums come from N=1 matmuls against a ones vector.
"""
import numpy as np


def _ensure_paths():
    import sys
    try:
        import concourse  # noqa: F401
        return
    except ImportError:
        pass
    for p in ("/opt/trn_rl_repo", "/root/.axon_site/_ro/trn_rl_repo"):
        if p not in sys.path:
            sys.path.insert(0, p)
    import concourse  # noqa: F401


B, S, DIN, DOUT = 2, 2048, 4096, 4096
A_SPLIT, B_SPLIT = 2, 4          # m-halves x n-quarters = 8 cores
M_SH = (B * S) // A_SPLIT        # 2048 rows per core
N_SH = DOUT // B_SPLIT           # 1024 cols per core
P = 128
KO = DIN // P                    # 32 k-tiles
MT = M_SH // P                   # 16 m-tiles
NT = N_SH // 512                 # 2 n-tiles of 512
NTILE = 512
XCH = 4                          # x load chunks per m-tile
XCW = DIN // XCH                 # 1024 columns per chunk


def _build(scale_f: float, zp_f: float):
    from contextlib import ExitStack
    import concourse.bass as bass
    import concourse.tile as tile
    from concourse import bacc, mybir
    from concourse.masks import make_identity
    from concourse.bass import ts

    f32 = mybir.dt.float32
    f32r = mybir.dt.float32r

    nc = bacc.Bacc("TRN2", target_bir_lowering=False, debug=False)

    xs = nc.dram_tensor("xs", [M_SH, DIN], f32, kind="ExternalInput")
    qs = nc.dram_tensor("qs", [DIN, N_SH], f32r, kind="ExternalInput")
    biass = nc.dram_tensor("biass", [N_SH], f32, kind="ExternalInput")
    ys = nc.dram_tensor("ys", [M_SH, N_SH], f32, kind="ExternalOutput")

    qs_t = qs.rearrange("(ko p) n -> p ko n", p=P)

    with tile.TileContext(nc) as tc, ExitStack() as ctx:
        const = ctx.enter_context(tc.tile_pool(name="const", bufs=1))
        qpool = ctx.enter_context(tc.tile_pool(name="qpool", bufs=1))
        xs_pool = ctx.enter_context(tc.tile_pool(name="xs_pool", bufs=2))
        xt_pool = ctx.enter_context(tc.tile_pool(name="xt_pool", bufs=2))
        y_pool = ctx.enter_context(tc.tile_pool(name="y_pool", bufs=3))
        small = ctx.enter_context(tc.tile_pool(name="small", bufs=2))
        mm_psum = ctx.enter_context(
            tc.tile_pool(name="mm_psum", bufs=4, space="PSUM"))
        tr_psum = ctx.enter_context(
            tc.tile_pool(name="tr_psum", bufs=4, space="PSUM"))

        ident = const.tile([P, P], f32)
        make_identity(nc, ident)
        bias_sb = const.tile([P, N_SH], f32)
        nc.sync.dma_start(bias_sb[:], biass[:].partition_broadcast(P))

        # q weight cache: [128, 32, 1024] f32r (128 KB/partition)
        qw = qpool.tile([P, KO, N_SH], f32r)
        for g in range(4):
            nc.sync.dma_start(qw[:, ts(g, KO // 4), :],
                              qs_t[:, ts(g, KO // 4), :])

        for mi in range(MT):
            # ---- load x m-panel, transpose it through the PE, and row-sum ----
            xT = xt_pool.tile([P, KO, P], f32r, tag="xT")
            rs4 = small.tile([P, XCH], f32, tag="rs4")
            for c in range(XCH):
                xst = xs_pool.tile([P, XCW], f32, tag="xst")
                nc.sync.dma_start(xst[:], xs[ts(mi, P), ts(c, XCW)])
                nc.vector.tensor_reduce(rs4[:, c:c + 1], xst[:],
                                        mybir.AxisListType.X,
                                        mybir.AluOpType.add)
                for j in range(XCH * 2):
                    ki = (KO // XCH) * c + j
                    tp = tr_psum.tile([P, P], f32, tag="tp")
                    nc.tensor.transpose(tp[:], xst[:, ts(j, P)], ident[:])
                    if ki % 2 == 0:
                        nc.vector.tensor_copy(out=xT[:, ki, :], in_=tp[:])
                    else:
                        nc.scalar.copy(out=xT[:, ki, :], in_=tp[:])

            # rs_term = -scale*zp * rowsum(x m-panel)
            rs_term = small.tile([P, 1], f32, tag="rs_term")
            nc.vector.tensor_reduce(rs_term[:], rs4[:],
                                    mybir.AxisListType.X,
                                    mybir.AluOpType.add)
            nc.vector.tensor_scalar_mul(rs_term[:], rs_term[:],
                                        -scale_f * zp_f)

            # ---- main matmuls ----
            for ni in range(NT):
                acc = mm_psum.tile([P, NTILE], f32, tag="acc")
                for ki in range(KO):
                    nc.tensor.matmul(acc[:], lhsT=xT[:, ki, :],
                                     rhs=qw[:, ki, ts(ni, NTILE)],
                                     start=(ki == 0), stop=(ki == KO - 1))
                y = y_pool.tile([P, NTILE], f32, tag="y")
                nc.vector.tensor_scalar(y[:], acc[:], scale_f, rs_term[:],
                                        mybir.AluOpType.mult,
                                        mybir.AluOpType.add)
                nc.vector.tensor_tensor(y[:], y[:], bias_sb[:, ts(ni, NTILE)],
                                        mybir.AluOpType.add)
                nc.sync.dma_start(ys[ts(mi, P), ts(ni, NTILE)], y[:])

    nc.compile()
    return nc


def kernel(x: np.ndarray, q_int_weight: np.ndarray, scale: np.ndarray,
           zero_point: np.ndarray, bias: np.ndarray) -> np.ndarray:
    _ensure_paths()
    from concourse.bass_utils import run_bass_kernel_spmd

    xf = np.ascontiguousarray(x.reshape(B * S, DIN).astype(np.float32))
    scale_f = float(np.asarray(scale).reshape(-1)[0])
    zp_f = float(np.asarray(zero_point).reshape(-1)[0])

    nc = _build(scale_f, zp_f)

    in_maps = []
    for c in range(8):
        mh, nq = divmod(c, B_SPLIT)
        in_maps.append({
            "xs": np.ascontiguousarray(xf[mh * M_SH:(mh + 1) * M_SH]),
            "qs": np.ascontiguousarray(
                q_int_weight[:, nq * N_SH:(nq + 1) * N_SH]).astype(np.float32),
            "biass": np.ascontiguousarray(bias[nq * N_SH:(nq + 1) * N_SH])
            .astype(np.float32),
        })

    res = run_bass_kernel_spmd(nc, in_maps, core_ids=list(range(8)))

    y = np.empty((B * S, DOUT), np.float32)
    for c in range(8):
        mh, nq = divmod(c, B_SPLIT)
        y[mh * M_SH:(mh + 1) * M_SH, nq * N_SH:(nq + 1) * N_SH] = \
            res.results[c]["ys"]
    return y.reshape(B, S, DOUT)


# revision 13
# speedup vs baseline: 1.2448x; 1.2448x over previous
"""TRN2 Bass kernel for FFQLinear: y = x @ ((q - zp) * scale) + bias.

x: [2, 2048, 4096] f32, q: [4096, 4096] int32 (values 0..255),
scale/zero_point: [1] f32, bias: [4096] f32 -> y: [2, 2048, 4096] f32.

Strategy (8 NeuronCores, 4x2 grid):
  - M (=B*S=4096) split in 4, N (=DOUT) split in 2.
  - Per core: x_shard [1024, 4096], q_shard [4096, 2048], out [1024, 2048].
  - Matmuls run in float32r (TF32-like, full PE rate, ~1e-4 abs/scale).
    q values (integers 0..255) are exact under f32r truncation, so the
    zero-point is handled exactly via a row-sum correction:
      y = scale * (x @ q) - (scale*zp) * rowsum(x) + bias
  - Phase 1: transpose the whole x shard through the PE (fp32 identity
    matmul) into a resident SBUF panel xT [4096k x 1024m]; row-sums on DVE.
  - Phase 2: stream q in [128,8,512] chunks, 8 concurrent PSUM accumulation
    groups (one per m-tile), fused scale/zp/bias epilogue on DVE.
"""
import numpy as np


def _ensure_paths():
    import sys
    try:
        import concourse  # noqa: F401
        return
    except ImportError:
        pass
    for p in ("/opt/trn_rl_repo", "/root/.axon_site/_ro/trn_rl_repo"):
        if p not in sys.path:
            sys.path.insert(0, p)
    import concourse  # noqa: F401


B, S, DIN, DOUT = 2, 2048, 4096, 4096
A_SPLIT, B_SPLIT = 4, 2          # m-quarters x n-halves = 8 cores
M_SH = (B * S) // A_SPLIT        # 1024 rows per core
N_SH = DOUT // B_SPLIT           # 2048 cols per core
P = 128
KO = DIN // P                    # 32 k-tiles
MT = M_SH // P                   # 8 m-tiles
NTILE = 512
NT = N_SH // NTILE               # 4 n-tiles of 512
KG = 8                           # k-tiles per q stream chunk
NG = KO // KG                    # 4 q chunks per n-tile
XCH = 4                          # x load chunks per m-tile
XCW = DIN // XCH                 # 1024 columns per chunk


def _build(scale_f: float, zp_f: float, reps: int = 1):
    from contextlib import ExitStack
    import concourse.bass as bass
    import concourse.tile as tile
    from concourse import bacc, mybir
    from concourse.masks import make_identity
    from concourse.bass import ts

    f32 = mybir.dt.float32
    f32r = mybir.dt.float32r

    nc = bacc.Bacc("TRN2", target_bir_lowering=False, debug=False)

    xs = nc.dram_tensor("xs", [M_SH, DIN], f32, kind="ExternalInput")
    qs = nc.dram_tensor("qs", [DIN, N_SH], f32r, kind="ExternalInput")
    biass = nc.dram_tensor("biass", [N_SH], f32, kind="ExternalInput")
    ys = nc.dram_tensor("ys", [M_SH, N_SH], f32, kind="ExternalOutput")

    qs_t = qs.rearrange("(ko p) n -> p ko n", p=P)

    with tile.TileContext(nc) as tc, ExitStack() as ctx:
        const = ctx.enter_context(tc.tile_pool(name="const", bufs=1))
        xt_pool = ctx.enter_context(tc.tile_pool(name="xt_pool", bufs=1))
        q_pool = ctx.enter_context(tc.tile_pool(name="q_pool", bufs=2))
        xs_pool = ctx.enter_context(tc.tile_pool(name="xs_pool", bufs=2))
        y_pool = ctx.enter_context(tc.tile_pool(name="y_pool", bufs=3))
        small = ctx.enter_context(tc.tile_pool(name="small", bufs=2))
        psum = ctx.enter_context(
            tc.tile_pool(name="psum", bufs=8, space="PSUM"))

        ident = const.tile([P, P], f32)
        make_identity(nc, ident)
        bias_sb = const.tile([P, N_SH], f32)
        nc.sync.dma_start(bias_sb[:], biass[:].partition_broadcast(P))

        def body():
            # ---- phase 1: transpose x shard into resident xT panel ----
            # xT[p, ko, m] = x[m, ko*128+p] for this core's m-range
            xT = xt_pool.tile([P, KO, M_SH], f32r, tag="xT")
            rs_all = const.tile([P, MT], f32, tag="rs_all")
            for mi in range(MT):
                rs4 = small.tile([P, XCH], f32, tag="rs4")
                for c in range(XCH):
                    xst = xs_pool.tile([P, XCW], f32, tag="xst")
                    nc.sync.dma_start(xst[:], xs[ts(mi, P), ts(c, XCW)])
                    nc.vector.tensor_reduce(rs4[:, c:c + 1], xst[:],
                                            mybir.AxisListType.X,
                                            mybir.AluOpType.add)
                    for j in range(KO // XCH):
                        ki = (KO // XCH) * c + j
                        tp = psum.tile([P, NTILE], f32, tag="acc")
                        nc.tensor.transpose(tp[:, :P], xst[:, ts(j, P)],
                                            ident[:])
                        if ki % 2 == 0:
                            nc.vector.tensor_copy(out=xT[:, ki, ts(mi, P)],
                                                  in_=tp[:, :P])
                        else:
                            nc.scalar.copy(out=xT[:, ki, ts(mi, P)],
                                           in_=tp[:, :P])
                # rowsum(x m-panel) * (-scale*zp)
                nc.vector.tensor_reduce(rs_all[:, mi:mi + 1], rs4[:],
                                        mybir.AxisListType.X,
                                        mybir.AluOpType.add)
            nc.vector.tensor_scalar_mul(rs_all[:], rs_all[:],
                                        -scale_f * zp_f)

            # ---- phase 2: stream q, 8 concurrent PSUM groups ----
            for ni in range(NT):
                accs = [psum.tile([P, NTILE], f32, tag="acc",
                                  name=f"acc_{ni}_{mi}")
                        for mi in range(MT)]
                for g in range(NG):
                    qg = q_pool.tile([P, KG, NTILE], f32r, tag="qg")
                    nc.sync.dma_start(
                        qg[:], qs_t[:, ts(g, KG), ts(ni, NTILE)])
                    for mi in range(MT):
                        for kj in range(KG):
                            ki = g * KG + kj
                            nc.tensor.matmul(
                                accs[mi][:], lhsT=xT[:, ki, ts(mi, P)],
                                rhs=qg[:, kj],
                                start=(ki == 0), stop=(ki == KO - 1))
                for mi in range(MT):
                    y = y_pool.tile([P, NTILE], f32, tag="y")
                    nc.vector.tensor_scalar(y[:], accs[mi][:], scale_f,
                                            rs_all[:, mi:mi + 1],
                                            mybir.AluOpType.mult,
                                            mybir.AluOpType.add)
                    nc.vector.tensor_tensor(y[:], y[:],
                                            bias_sb[:, ts(ni, NTILE)],
                                            mybir.AluOpType.add)
                    nc.sync.dma_start(ys[ts(mi, P), ts(ni, NTILE)], y[:])

        if reps == 1:
            body()
        else:
            with tc.For_i(0, reps, 1):
                body()

    nc.compile()
    return nc


def kernel(x: np.ndarray, q_int_weight: np.ndarray, scale: np.ndarray,
           zero_point: np.ndarray, bias: np.ndarray) -> np.ndarray:
    _ensure_paths()
    from concourse.bass_utils import run_bass_kernel_spmd

    xf = np.ascontiguousarray(x.reshape(B * S, DIN).astype(np.float32))
    scale_f = float(np.asarray(scale).reshape(-1)[0])
    zp_f = float(np.asarray(zero_point).reshape(-1)[0])

    nc = _build(scale_f, zp_f)

    in_maps = []
    for c in range(8):
        mq, nh = divmod(c, B_SPLIT)
        in_maps.append({
            "xs": np.ascontiguousarray(xf[mq * M_SH:(mq + 1) * M_SH]),
            "qs": np.ascontiguousarray(
                q_int_weight[:, nh * N_SH:(nh + 1) * N_SH]).astype(np.float32),
            "biass": np.ascontiguousarray(bias[nh * N_SH:(nh + 1) * N_SH])
            .astype(np.float32),
        })

    res = run_bass_kernel_spmd(nc, in_maps, core_ids=list(range(8)))

    y = np.empty((B * S, DOUT), np.float32)
    for c in range(8):
        mq, nh = divmod(c, B_SPLIT)
        y[mq * M_SH:(mq + 1) * M_SH, nh * N_SH:(nh + 1) * N_SH] = \
            res.results[c]["ys"]
    return y.reshape(B, S, DOUT)
